# revision 21
# baseline (speedup 1.0000x reference)
"""MultiHeadAttention forward on 8 Trainium2 NeuronCores (Bass/Tile), v2.

Problem (hardcoded): B=2, S=2048, D=1024, H=16, HD=64.
  qkv = x @ w_qkv.T + b_qkv ; per-head attention with softmax(q k^T/8 + mask);
  out = values @ w_out.T + b_out.

Sharding: tensor-parallel over heads -- core c owns heads {2c, 2c+1}
(value dims 128c..128c+127).  Each core computes its 2 heads end-to-end and
a partial output projection in bf16; the host sums the 8 partials and adds
the bias constant (b_out + b_v @ w_out.T, exact because softmax rows sum
to 1).

v2 design notes (vs the v1 baseline this evolved from):
 - Everything is bf16 on the wire and in SBUF (x, q/k, v, probs, vals, w):
   halves DMA traffic and DVE element counts; matmuls run 1 cyc/row with
   FWL weight loads.  PSUM accumulation stays f32.
 - x is loaded once (16 x 512KB DMAs, batch-0 chunks first) and stays
   SBUF-resident; the ones row/columns come from memsets, not DMAs.
   Weight DMAs go out on the scalar-engine HWDGE queue in 4 batched
   transfers so the sync queue starts on x immediately (v1 spent ~70us
   serially issuing 65 tiny DMAs before any compute).
 - Scores are computed transposed (S^T = K^T.T @ Q^T per head) so exp runs
   on ScalarE straight out of PSUM; the two heads' score matmuls sit on
   disjoint PE row groups (partitions 0-63 / 64-127) and run concurrently.
 - V carries ones columns ([64v|32ones] for head 0, [32ones|64v] for head
   1, written at PSUM base 0 / 32) so one AV matmul per head yields both
   values^T and the softmax denominator l; head 1's values land on PSUM
   partitions 64:128, so the two heads' normalized values form a single
   [128, tq] vals tile and the output projection is one K=128 matmul per
   128-token block (v1 needed two K=64 matmuls).
 - l sits on a single partition row; a direct DVE reciprocal on the [1,512]
   row replaces v1's 32x32 transpose dance, and a K=1 matmul against a
   memset ones row broadcasts 1/l across partitions for the normalize
   multiply.
 - The emission order software-pipelines the whole kernel: Phase A (b=0)
   is interleaved with the first attention block's chunk loop, and a
   deferred-thunk queue drips Phase A (b=1) and the previous block's
   output projection into later chunk loops, so the PE never idles long
   enough for the HAM clock gate to re-throttle it to 1.2 GHz (v1 ran
   ~59% of its span at half clock) while ScalarE exp (the Phase B floor,
   ~1.15us per 128x1024 chunk) stays saturated.
"""
import sys
if "/opt/trn_rl_repo" not in sys.path:
    sys.path.insert(0, "/opt/trn_rl_repo")
import numpy as np

B, S, D, H = 2, 2048, 1024, 16
HD = D // H           # 64
NCORES = 8
T = B * S             # 4096 tokens
NB = S // 512         # 4 tq blocks per batch
NCH = S // 128        # 16 kpos chunks per batch

_CACHE = {}


def build_nc(use_mask: bool, reps: int = 1):
    """Build + compile the per-core Bass program (SPMD-identical)."""
    import concourse.bacc as bacc
    import concourse.tile as tile
    from concourse import mybir

    f32 = mybir.dt.float32
    f32r = mybir.dt.float32r
    bf16 = mybir.dt.bfloat16
    EXP = mybir.ActivationFunctionType.Exp
    MULT = mybir.AluOpType.mult

    nc = bacc.Bacc("TRN2", target_bir_lowering=False, debug=False,
                   num_devices=NCORES)

    xTb = nc.dram_tensor("xTb", (128, 8, B, S), bf16, kind="ExternalInput")
    wqk = nc.dram_tensor("wqk", (128, 8, 256), bf16, kind="ExternalInput")
    wv = nc.dram_tensor("wv", (128, 8, 128), bf16, kind="ExternalInput")
    wo = nc.dram_tensor("wo", (128, D), bf16, kind="ExternalInput")
    bqk = nc.dram_tensor("bqk", (128, 2), f32, kind="ExternalInput")
    if use_mask:
        maskT = nc.dram_tensor("maskT", (B, S, S), f32r, kind="ExternalInput")
        ident = nc.dram_tensor("ident", (128, 128), f32r, kind="ExternalInput")
    out = nc.dram_tensor("out", (T, D), bf16, kind="ExternalOutput")

    with tile.TileContext(nc) as tc:
        with tc.tile_pool(name="sbp", bufs=1) as sbp, \
             tc.tile_pool(name="ptp", bufs=4) as ptp, \
             tc.tile_pool(name="otp", bufs=2) as otp, \
             tc.tile_pool(name="rlp", bufs=2) as rlp, \
             tc.tile_pool(name="bcp", bufs=2) as bcp, \
             tc.tile_pool(name="mkp", bufs=4) as mkp, \
             tc.tile_pool(name="mmp", bufs=2, space="PSUM") as mmp, \
             tc.tile_pool(name="scp", bufs=2, space="PSUM") as scp, \
             tc.tile_pool(name="avp", bufs=2, space="PSUM") as avp:

            # --- persistent SBUF tensors (separate tiles per block so the
            # Tile dependency tracker never sees false cross-block deps) ---
            xb = [[sbp.tile([128, S], bf16, name=f"xb_{cc}_{b}")
                   for b in range(B)] for cc in range(8)]
            qkt = [[sbp.tile([128, 2, 512], bf16, name=f"qkt_{b}_{tb}")
                    for tb in range(4)] for b in range(B)]
            vext = [[sbp.tile([128, 2, 4, 128], bf16, name=f"vext_{b}_{tb}")
                     for tb in range(4)] for b in range(B)]
            vals = [[sbp.tile([128, 512], bf16, name=f"vals_{b}_{tqb}")
                     for tqb in range(NB)] for b in range(B)]
            wqk_sb = sbp.tile([128, 8, 256], bf16, name="wqk_sb")
            wv_sb = sbp.tile([128, 8, 128], bf16, name="wv_sb")
            wo_sb = sbp.tile([128, D], bf16, name="wo_sb")
            bqk_sb = sbp.tile([128, 2], f32, name="bqk_sb")
            ones_f32 = sbp.tile([65, 128], f32, name="ones_f32")
            ones_sb = sbp.tile([65, 128], f32r, name="ones_sb")
            if use_mask:
                id_sb = sbp.tile([128, 128], f32r, name="id_sb")

            # weight loads on the scalar-engine HWDGE queue (parallel with
            # the x loads below on the sync queue)
            nc.scalar.dma_start(wqk_sb, wqk[:, :, :])
            nc.scalar.dma_start(wv_sb, wv[:, :, :])
            nc.scalar.dma_start(wo_sb, wo[:, :])
            nc.scalar.dma_start(bqk_sb, bqk[:, :])
            if use_mask:
                nc.scalar.dma_start(id_sb, ident[:, :])

            nc.vector.memset(ones_f32, 1.0)
            nc.vector.tensor_copy(ones_sb, ones_f32)
            for b in range(B):
                for tb in range(4):
                    nc.vector.memset(vext[b][tb][:, 0, :, 64:128], 1.0)
                    nc.vector.memset(vext[b][tb][:, 1, :, 0:64], 1.0)

            for rep in range(reps):
                # x loads: batch 0's feature chunks first so Phase A can
                # start as soon as chunk (0,0) lands
                if rep == 0:
                    for b in range(B):
                        for cc in range(8):
                            nc.sync.dma_start(xb[cc][b], xTb[:, cc, b, :])

                deferred = []

                def pop_def(n):
                    for _ in range(min(n, len(deferred))):
                        deferred.pop(0)()

                def qk_group(b, tb, m):
                    t0 = 512 * tb
                    cell = {}

                    def start():
                        cell["acc"] = mmp.tile(
                            [128, 512], f32, tag="mm",
                            name=f"qk_{rep}_{b}_{tb}_{m}")

                    def mm(cc):
                        nc.tensor.matmul(
                            cell["acc"],
                            wqk_sb[:, cc, 128 * m:128 * m + 128],
                            xb[cc][b][:, t0:t0 + 512],
                            start=(cc == 0), stop=(cc == 7))

                    def fin():
                        nc.vector.tensor_scalar_add(
                            qkt[b][tb][:, m, :], cell["acc"],
                            bqk_sb[:, m:m + 1])
                    return start, mm, fin

                def v_group(b, tb, u):
                    t0 = 512 * tb
                    cell = {}

                    def start():
                        cell["vp"] = mmp.tile(
                            [128, 512], f32, tag="mm",
                            name=f"vp_{rep}_{b}_{tb}_{u}")

                    def mm(cc):
                        nc.tensor.matmul(
                            cell["vp"][:, 0:128],
                            xb[cc][b][:, t0 + 128 * u:t0 + 128 * u + 128],
                            wv_sb[:, cc, :],
                            start=(cc == 0), stop=(cc == 7))

                    def fin():
                        nc.vector.tensor_copy(
                            vext[b][tb][:, 0, u, 0:64], cell["vp"][:, 0:64])
                        nc.vector.tensor_copy(
                            vext[b][tb][:, 1, u, 64:128], cell["vp"][:, 64:128])
                    return start, mm, fin

                def run_group(g):
                    start, mm, fin = g
                    start()
                    for cc in range(8):
                        mm(cc)
                    fin()

                def emit_tb_deferred(b, tb):
                    """Phase A for one token block as small deferred thunks."""
                    groups = [qk_group(b, tb, 0), qk_group(b, tb, 1)] + \
                             [v_group(b, tb, u) for u in range(4)]
                    for (start, mm, fin) in groups:
                        for cc0 in range(0, 8, 2):
                            def th(start=start, mm=mm, cc0=cc0):
                                if cc0 == 0:
                                    start()
                                mm(cc0)
                                mm(cc0 + 1)
                            deferred.append(th)
                        deferred.append(fin)

                def emit_sc_exp(b, tqb, c):
                    sc = scp.tile([128, 1024], f32, tag="sc",
                                  name=f"sc_{rep}_{b}_{tqb}_{c}")
                    q_aps = [qkt[b][tqb][64 * h:64 * h + 64, 0, :]
                             for h in range(2)]
                    for h in range(2):
                        k_ap = qkt[b][c // 4][64 * h:64 * h + 64, 1,
                                             128 * (c % 4):128 * (c % 4) + 128]
                        nc.tensor.matmul(sc[:, 512 * h:512 * h + 512],
                                         k_ap, q_aps[h],
                                         start=True, stop=(not use_mask))
                    if use_mask:
                        mt = mkp.tile([128, 512], f32r, tag="mk",
                                      name=f"mk_{rep}_{b}_{tqb}_{c}")
                        nc.sync.dma_start(
                            mt, maskT[b, 128 * c:128 * c + 128,
                                      512 * tqb:512 * tqb + 512])
                        for h in range(2):
                            nc.tensor.matmul(sc[:, 512 * h:512 * h + 512],
                                             id_sb, mt, start=False, stop=True)
                    pt = ptp.tile([128, 1024], bf16, tag="pt",
                                  name=f"pt_{rep}_{b}_{tqb}_{c}")
                    nc.scalar.activation(pt, sc, EXP)
                    return pt

                def emit_av(b, tqb, c, pt, avs):
                    for h in range(2):
                        nc.tensor.matmul(avs[h][:, :],
                                         vext[b][c // 4][:, h, c % 4, :],
                                         pt[:, 512 * h:512 * h + 512],
                                         start=(c == 0), stop=(c == NCH - 1))

                def emit_normalize(b, tqb, avs):
                    for h in range(2):
                        av = avs[h]
                        lrow = 64 if h == 0 else 32   # the denominator row
                        vlo = 0 if h == 0 else 64     # values partition base
                        rlf = rlp.tile([65, 512], f32, tag="rlf",
                                       name=f"rlf_{rep}_{b}_{tqb}_{h}")
                        nc.vector.reciprocal(rlf[lrow:lrow + 1, :],
                                             av[lrow:lrow + 1, :])
                        rl = rlp.tile([65, 512], f32r, tag="rl",
                                      name=f"rl_{rep}_{b}_{tqb}_{h}")
                        nc.vector.tensor_copy(rl[lrow:lrow + 1, :],
                                              rlf[lrow:lrow + 1, :])
                        bct = mmp.tile([128, 512], f32, tag="mm",
                                       name=f"bc_{rep}_{b}_{tqb}_{h}")
                        nc.tensor.matmul(bct[:, :],
                                         ones_sb[lrow:lrow + 1, :],
                                         rl[lrow:lrow + 1, :],
                                         start=True, stop=True)
                        bcs = bcp.tile([128, 512], f32, tag="bcs",
                                       name=f"bcs_{rep}_{b}_{tqb}_{h}")
                        nc.vector.tensor_copy(bcs[vlo:vlo + 64, :],
                                              bct[vlo:vlo + 64, :])
                        nc.vector.tensor_tensor(
                            vals[b][tqb][vlo:vlo + 64, :],
                            av[vlo:vlo + 64, :], bcs[vlo:vlo + 64, :], MULT)

                def defer_phase_c(b, tqb):
                    t0g = S * b + 512 * tqb
                    cell = {}

                    def oalloc():
                        cell["ot"] = otp.tile([128, 4, D], bf16, tag="ot",
                                              name=f"ot_{rep}_{b}_{tqb}")
                    for u in range(4):
                        for nb2 in range(2):
                            def th(u=u, nb2=nb2):
                                if u == 0 and nb2 == 0:
                                    oalloc()
                                op = mmp.tile([128, 512], f32, tag="mm",
                                              name=f"op_{rep}_{b}_{tqb}_{u}_{nb2}")
                                nc.tensor.matmul(
                                    op, vals[b][tqb][:, 128 * u:128 * u + 128],
                                    wo_sb[:, 512 * nb2:512 * nb2 + 512],
                                    start=True, stop=True)
                                nc.vector.tensor_copy(
                                    cell["ot"][:, u, 512 * nb2:512 * nb2 + 512],
                                    op)
                            deferred.append(th)

                    def dth():
                        nc.sync.dma_start(
                            out[t0g:t0g + 512, :].rearrange(
                                "(u p) f -> p u f", p=128), cell["ot"])
                    deferred.append(dth)

                def emit_tqb(b, tqb):
                    if b == 1 and tqb == 0:
                        # batch 1's attention reads qkt[1]/vext[1] produced by
                        # the deferred Phase A(b=1) thunks -- every one of
                        # them must be EMITTED (program order = dependency
                        # order for Tile) before these chunks are.
                        pop_def(len(deferred))
                    avs = [avp.tile([128, 512], f32, tag="av",
                                    name=f"av_{rep}_{b}_{tqb}_{h}")
                           for h in range(2)]
                    for c in range(NCH):
                        pt = emit_sc_exp(b, tqb, c)
                        emit_av(b, tqb, c, pt, avs)
                        pop_def(3)
                    emit_normalize(b, tqb, avs)
                    defer_phase_c(b, tqb)

                # ---- head: Phase A (b=0) interleaved with (0, tqb0):
                # per chunk, the vp group lands between the score matmuls and
                # the (exp-gated) AV matmuls so ScalarE starts at ~t_qk(tb0)
                # and the PE never queues a long vp burst ahead of scores.
                run_group(qk_group(0, 0, 0))
                run_group(qk_group(0, 0, 1))
                avs00 = [avp.tile([128, 512], f32, tag="av",
                                  name=f"av_{rep}_0_0_{h}")
                         for h in range(2)]
                for c in range(NCH):
                    if c % 4 == 0 and c > 0:
                        run_group(qk_group(0, c // 4, 0))
                        run_group(qk_group(0, c // 4, 1))
                    pt = emit_sc_exp(0, 0, c)
                    run_group(v_group(0, c // 4, c % 4))
                    emit_av(0, 0, c, pt, avs00)
                emit_normalize(0, 0, avs00)
                defer_phase_c(0, 0)
                # ---- steady state ----
                for b in range(B):
                    for tqb in range(NB):
                        if b == 0 and tqb == 0:
                            continue
                        if b == 0 and tqb == 1:
                            for tb in range(4):
                                emit_tb_deferred(1, tb)
                        emit_tqb(b, tqb)
                while deferred:
                    pop_def(8)
    nc.compile()
    return nc


def make_in_maps(mha_x, self_mask, w_qkv, b_qkv, w_out, b_out, use_mask):
    """Host-side sharding / layout prep. Returns (in_maps, host_bias)."""
    import ml_dtypes
    bf = np.dtype(ml_dtypes.bfloat16)
    x = np.asarray(mha_x, np.float32).reshape(T, D)
    xT_np = np.ascontiguousarray(x.T)                   # [D, T]
    # [128, 8(cc), B, S] with xTb[p, cc, b, t] = x[2048b + t, 128cc + p]
    xTb_np = np.ascontiguousarray(
        xT_np.reshape(8, 128, B, S).transpose(1, 0, 2, 3).astype(bf))
    scale = 1.0 / np.sqrt(np.float32(HD))               # 1/8
    wqkv = np.asarray(w_qkv, np.float32)
    bqkv = np.asarray(b_qkv, np.float32)
    wout = np.asarray(w_out, np.float32)
    bout = np.asarray(b_out, np.float32)

    # reference packs w_qkv rows as [H, (q,k,v), HD]: head h's q rows are
    # wqkv[192h:192h+64], k rows +64, v rows +128.
    wq_rows = lambda h: wqkv[192 * h:192 * h + 64, :]
    wk_rows = lambda h: wqkv[192 * h + 64:192 * h + 128, :]
    wv_rows = lambda h: wqkv[192 * h + 128:192 * h + 192, :]
    bq_of = lambda h: bqkv[192 * h:192 * h + 64]
    bk_of = lambda h: bqkv[192 * h + 64:192 * h + 128]
    bv_of = lambda h: bqkv[192 * h + 128:192 * h + 192]

    in_maps = []
    for c in range(NCORES):
        h0, h1 = 2 * c, 2 * c + 1
        wq = np.concatenate([wq_rows(h0), wq_rows(h1)], 0) * scale
        wk = np.concatenate([wk_rows(h0), wk_rows(h1)], 0)
        wvm = np.concatenate([wv_rows(h0), wv_rows(h1)], 0)
        wqkT = np.concatenate([wq, wk], 0).T            # [1024, 256]
        wvT = wvm.T                                     # [1024, 128]
        m = {
            "xTb": xTb_np,
            "wqk": np.ascontiguousarray(
                wqkT.reshape(8, 128, 256).transpose(1, 0, 2).astype(bf)),
            "wv": np.ascontiguousarray(
                wvT.reshape(8, 128, 128).transpose(1, 0, 2).astype(bf)),
            "wo": np.ascontiguousarray(
                wout[:, 128 * c:128 * c + 128].T.astype(bf)),
            "bqk": np.ascontiguousarray(
                np.stack([np.concatenate([bq_of(h0), bq_of(h1)]) * scale,
                          np.concatenate([bk_of(h0), bk_of(h1)])], 1)),
        }
        if use_mask:
            m["maskT"] = np.ascontiguousarray(
                np.asarray(self_mask, np.float32).transpose(0, 2, 1))
            m["ident"] = np.eye(128, dtype=np.float32)
        in_maps.append(m)

    b_v_full = np.concatenate([bv_of(h) for h in range(H)])
    host_bias = b_v_full @ wout.T + bout                # [D], exact
    return in_maps, host_bias


def kernel(**inputs):
    from concourse.bass_utils import run_bass_kernel_spmd
    self_mask = np.asarray(inputs["self_mask"], np.float32)
    use_mask = bool(np.any(self_mask))
    key = ("nc", use_mask)
    if key not in _CACHE:
        _CACHE[key] = build_nc(use_mask)
    nc = _CACHE[key]
    in_maps, host_bias = make_in_maps(
        inputs["mha_x"], self_mask, inputs["w_qkv"], inputs["b_qkv"],
        inputs["w_out"], inputs["b_out"], use_mask)
    res = run_bass_kernel_spmd(nc, in_maps, core_ids=list(range(NCORES)))
    acc = np.zeros((T, D), np.float32)
    for c in range(NCORES):
        acc += np.asarray(res.results[c]["out"], np.float32)
    acc += host_bias[None, :]
    return acc.reshape(B, S, D)


# revision 25
# speedup vs baseline: 1.0015x; 1.0015x over previous
"""MultiHeadAttention forward on 8 Trainium2 NeuronCores (Bass/Tile), v2.

Problem (hardcoded): B=2, S=2048, D=1024, H=16, HD=64.
  qkv = x @ w_qkv.T + b_qkv ; per-head attention with softmax(q k^T/8 + mask);
  out = values @ w_out.T + b_out.

Sharding: tensor-parallel over heads -- core c owns heads {2c, 2c+1}
(value dims 128c..128c+127).  Each core computes its 2 heads end-to-end and
a partial output projection in bf16; the host sums the 8 partials and adds
the bias constant (b_out + b_v @ w_out.T, exact because softmax rows sum
to 1).

v2 design notes (vs the v1 baseline this evolved from):
 - Everything is bf16 on the wire and in SBUF (x, q/k, v, probs, vals, w):
   halves DMA traffic and DVE element counts; matmuls run 1 cyc/row with
   FWL weight loads.  PSUM accumulation stays f32.
 - x is loaded once (16 x 512KB DMAs, batch-0 chunks first) and stays
   SBUF-resident; the ones row/columns come from memsets, not DMAs.
   Weight DMAs go out on the scalar-engine HWDGE queue in 4 batched
   transfers so the sync queue starts on x immediately (v1 spent ~70us
   serially issuing 65 tiny DMAs before any compute).
 - Scores are computed transposed (S^T = K^T.T @ Q^T per head) so exp runs
   on ScalarE straight out of PSUM; the two heads' score matmuls sit on
   disjoint PE row groups (partitions 0-63 / 64-127) and run concurrently.
 - V carries ones columns ([64v|32ones] for head 0, [32ones|64v] for head
   1, written at PSUM base 0 / 32) so one AV matmul per head yields both
   values^T and the softmax denominator l; head 1's values land on PSUM
   partitions 64:128, so the two heads' normalized values form a single
   [128, tq] vals tile and the output projection is one K=128 matmul per
   128-token block (v1 needed two K=64 matmuls).
 - l sits on a single partition row; a direct DVE reciprocal on the [1,512]
   row replaces v1's 32x32 transpose dance, and a K=1 matmul against a
   memset ones row broadcasts 1/l across partitions for the normalize
   multiply.
 - The emission order software-pipelines the whole kernel: Phase A (b=0)
   is interleaved with the first attention block's chunk loop, and a
   deferred-thunk queue drips Phase A (b=1) and the previous block's
   output projection into later chunk loops, so the PE never idles long
   enough for the HAM clock gate to re-throttle it to 1.2 GHz (v1 ran
   ~59% of its span at half clock) while ScalarE exp (the Phase B floor,
   ~1.15us per 128x1024 chunk) stays saturated.
"""
import sys
if "/opt/trn_rl_repo" not in sys.path:
    sys.path.insert(0, "/opt/trn_rl_repo")
import numpy as np

B, S, D, H = 2, 2048, 1024, 16
HD = D // H           # 64
NCORES = 8
T = B * S             # 4096 tokens
NB = S // 512         # 4 tq blocks per batch
NCH = S // 128        # 16 kpos chunks per batch

_CACHE = {}


def build_nc(use_mask: bool, reps: int = 1):
    """Build + compile the per-core Bass program (SPMD-identical)."""
    import concourse.bacc as bacc
    import concourse.tile as tile
    from concourse import mybir

    f32 = mybir.dt.float32
    f32r = mybir.dt.float32r
    bf16 = mybir.dt.bfloat16
    EXP = mybir.ActivationFunctionType.Exp
    MULT = mybir.AluOpType.mult

    nc = bacc.Bacc("TRN2", target_bir_lowering=False, debug=False,
                   num_devices=NCORES)

    xTb = nc.dram_tensor("xTb", (128, 8, B, S), bf16, kind="ExternalInput")
    wqk = nc.dram_tensor("wqk", (128, 8, 256), bf16, kind="ExternalInput")
    wv = nc.dram_tensor("wv", (128, 8, 128), bf16, kind="ExternalInput")
    wo = nc.dram_tensor("wo", (128, D), bf16, kind="ExternalInput")
    bqk = nc.dram_tensor("bqk", (128, 2), f32, kind="ExternalInput")
    if use_mask:
        maskT = nc.dram_tensor("maskT", (B, S, S), f32r, kind="ExternalInput")
        ident = nc.dram_tensor("ident", (128, 128), f32r, kind="ExternalInput")
    out = nc.dram_tensor("out", (T, D), bf16, kind="ExternalOutput")

    with tile.TileContext(nc) as tc:
        with tc.tile_pool(name="sbp", bufs=1) as sbp, \
             tc.tile_pool(name="ptp", bufs=4) as ptp, \
             tc.tile_pool(name="otp", bufs=2) as otp, \
             tc.tile_pool(name="rlp", bufs=6) as rlp, \
             tc.tile_pool(name="bcp", bufs=6) as bcp, \
             tc.tile_pool(name="mkp", bufs=4) as mkp, \
             tc.tile_pool(name="mmp", bufs=2, space="PSUM") as mmp, \
             tc.tile_pool(name="scp", bufs=2, space="PSUM") as scp, \
             tc.tile_pool(name="avp", bufs=2, space="PSUM") as avp:

            # --- persistent SBUF tensors (separate tiles per block so the
            # Tile dependency tracker never sees false cross-block deps) ---
            xb = [[sbp.tile([128, S], bf16, name=f"xb_{cc}_{b}")
                   for b in range(B)] for cc in range(8)]
            qkt = [[sbp.tile([128, 2, 512], bf16, name=f"qkt_{b}_{tb}")
                    for tb in range(4)] for b in range(B)]
            vext = [[sbp.tile([128, 2, 4, 128], bf16, name=f"vext_{b}_{tb}")
                     for tb in range(4)] for b in range(B)]
            vals = [[sbp.tile([128, 512], bf16, name=f"vals_{b}_{tqb}")
                     for tqb in range(NB)] for b in range(B)]
            wqk_sb = sbp.tile([128, 8, 256], bf16, name="wqk_sb")
            wv_sb = sbp.tile([128, 8, 128], bf16, name="wv_sb")
            wo_sb = sbp.tile([128, D], bf16, name="wo_sb")
            bqk_sb = sbp.tile([128, 2], f32, name="bqk_sb")
            ones_f32 = sbp.tile([65, 128], f32, name="ones_f32")
            ones_sb = sbp.tile([65, 128], f32r, name="ones_sb")
            if use_mask:
                id_sb = sbp.tile([128, 128], f32r, name="id_sb")

            # weight loads on the scalar-engine HWDGE queue (parallel with
            # the x loads below on the sync queue)
            nc.scalar.dma_start(wqk_sb, wqk[:, :, :])
            nc.scalar.dma_start(wv_sb, wv[:, :, :])
            nc.scalar.dma_start(wo_sb, wo[:, :])
            nc.scalar.dma_start(bqk_sb, bqk[:, :])
            if use_mask:
                nc.scalar.dma_start(id_sb, ident[:, :])

            nc.vector.memset(ones_f32, 1.0)
            nc.vector.tensor_copy(ones_sb, ones_f32)
            for b in range(B):
                for tb in range(4):
                    nc.vector.memset(vext[b][tb][:, 0, :, 64:128], 1.0)
                    nc.vector.memset(vext[b][tb][:, 1, :, 0:64], 1.0)

            for rep in range(reps):
                # x loads: batch 0's feature chunks first so Phase A can
                # start as soon as chunk (0,0) lands
                if rep == 0:
                    for b in range(B):
                        for cc in range(8):
                            nc.sync.dma_start(xb[cc][b], xTb[:, cc, b, :])

                deferred = []

                def pop_def(n):
                    for _ in range(min(n, len(deferred))):
                        deferred.pop(0)()

                def qk_group(b, tb, m):
                    t0 = 512 * tb
                    cell = {}

                    def start():
                        cell["acc"] = mmp.tile(
                            [128, 512], f32, tag="mm",
                            name=f"qk_{rep}_{b}_{tb}_{m}")

                    def mm(cc):
                        nc.tensor.matmul(
                            cell["acc"],
                            wqk_sb[:, cc, 128 * m:128 * m + 128],
                            xb[cc][b][:, t0:t0 + 512],
                            start=(cc == 0), stop=(cc == 7))

                    def fin():
                        nc.vector.tensor_scalar_add(
                            qkt[b][tb][:, m, :], cell["acc"],
                            bqk_sb[:, m:m + 1])
                    return start, mm, fin

                def v_group(b, tb, u):
                    t0 = 512 * tb
                    cell = {}

                    def start():
                        cell["vp"] = mmp.tile(
                            [128, 512], f32, tag="mm",
                            name=f"vp_{rep}_{b}_{tb}_{u}")

                    def mm(cc):
                        nc.tensor.matmul(
                            cell["vp"][:, 0:128],
                            xb[cc][b][:, t0 + 128 * u:t0 + 128 * u + 128],
                            wv_sb[:, cc, :],
                            start=(cc == 0), stop=(cc == 7))

                    def fin():
                        nc.vector.tensor_copy(
                            vext[b][tb][:, 0, u, 0:64], cell["vp"][:, 0:64])
                        nc.vector.tensor_copy(
                            vext[b][tb][:, 1, u, 64:128], cell["vp"][:, 64:128])
                    return start, mm, fin

                def run_group(g):
                    start, mm, fin = g
                    start()
                    for cc in range(8):
                        mm(cc)
                    fin()

                def emit_tb_deferred(b, tb):
                    """Phase A for one token block as small deferred thunks."""
                    groups = [qk_group(b, tb, 0), qk_group(b, tb, 1)] + \
                             [v_group(b, tb, u) for u in range(4)]
                    for (start, mm, fin) in groups:
                        for cc0 in range(0, 8, 2):
                            def th(start=start, mm=mm, cc0=cc0):
                                if cc0 == 0:
                                    start()
                                mm(cc0)
                                mm(cc0 + 1)
                            deferred.append(th)
                        deferred.append(fin)

                def emit_sc_exp(b, tqb, c):
                    sc = scp.tile([128, 1024], f32, tag="sc",
                                  name=f"sc_{rep}_{b}_{tqb}_{c}")
                    q_aps = [qkt[b][tqb][64 * h:64 * h + 64, 0, :]
                             for h in range(2)]
                    for h in range(2):
                        k_ap = qkt[b][c // 4][64 * h:64 * h + 64, 1,
                                             128 * (c % 4):128 * (c % 4) + 128]
                        nc.tensor.matmul(sc[:, 512 * h:512 * h + 512],
                                         k_ap, q_aps[h],
                                         start=True, stop=(not use_mask))
                    if use_mask:
                        mt = mkp.tile([128, 512], f32r, tag="mk",
                                      name=f"mk_{rep}_{b}_{tqb}_{c}")
                        nc.sync.dma_start(
                            mt, maskT[b, 128 * c:128 * c + 128,
                                      512 * tqb:512 * tqb + 512])
                        for h in range(2):
                            nc.tensor.matmul(sc[:, 512 * h:512 * h + 512],
                                             id_sb, mt, start=False, stop=True)
                    pt = ptp.tile([128, 1024], bf16, tag="pt",
                                  name=f"pt_{rep}_{b}_{tqb}_{c}")
                    nc.scalar.activation(pt, sc, EXP)
                    return pt

                def emit_av(b, tqb, c, pt, avs):
                    for h in range(2):
                        nc.tensor.matmul(avs[h][:, :],
                                         vext[b][c // 4][:, h, c % 4, :],
                                         pt[:, 512 * h:512 * h + 512],
                                         start=(c == 0), stop=(c == NCH - 1))

                def emit_normalize(b, tqb, avs):
                    """Deferred softmax normalization.  The first thunk copies
                    everything needed out of the PSUM av tiles (releasing
                    their slots for the next block's AV accumulation with no
                    PE->DVE->PE cycle); the rest float freely in DVE slack."""
                    def evac_part(h, cell):
                        av = avs[h]
                        lrow = 64 if h == 0 else 32   # the denominator row
                        vlo = 0 if h == 0 else 64     # values partition base
                        avu = bcp.tile([128, 512], bf16, tag="avu",
                                       name=f"avu_{rep}_{b}_{tqb}_{h}")
                        nc.vector.tensor_copy(avu[vlo:vlo + 64, :],
                                              av[vlo:vlo + 64, :])
                        lsb = rlp.tile([65, 512], f32, tag="lsb",
                                       name=f"lsb_{rep}_{b}_{tqb}_{h}")
                        nc.vector.tensor_copy(lsb[lrow:lrow + 1, :],
                                              av[lrow:lrow + 1, :])
                        cell["avu"], cell["lsb"] = avu, lsb

                    def recip_part(h, cell):
                        lrow = 64 if h == 0 else 32
                        lsb = cell["lsb"]
                        rlf = rlp.tile([65, 512], f32, tag="rlf",
                                       name=f"rlf_{rep}_{b}_{tqb}_{h}")
                        nc.vector.reciprocal(rlf[lrow:lrow + 1, :],
                                             lsb[lrow:lrow + 1, :])
                        rl = rlp.tile([65, 512], f32r, tag="rl",
                                      name=f"rl_{rep}_{b}_{tqb}_{h}")
                        nc.vector.tensor_copy(rl[lrow:lrow + 1, :],
                                              rlf[lrow:lrow + 1, :])
                        cell["rl"] = rl

                    def bc_part(h, cell):
                        lrow = 64 if h == 0 else 32
                        vlo = 0 if h == 0 else 64
                        bct = mmp.tile([128, 512], f32, tag="mm",
                                       name=f"bc_{rep}_{b}_{tqb}_{h}")
                        nc.tensor.matmul(bct[:, :],
                                         ones_sb[lrow:lrow + 1, :],
                                         cell["rl"][lrow:lrow + 1, :],
                                         start=True, stop=True)
                        bcs = bcp.tile([128, 512], bf16, tag="bcs",
                                       name=f"bcs_{rep}_{b}_{tqb}_{h}")
                        nc.vector.tensor_copy(bcs[vlo:vlo + 64, :],
                                              bct[vlo:vlo + 64, :])
                        nc.vector.tensor_tensor(
                            vals[b][tqb][vlo:vlo + 64, :],
                            cell["avu"][vlo:vlo + 64, :],
                            bcs[vlo:vlo + 64, :], MULT)

                    for h in range(2):
                        cell = {}
                        deferred.append(
                            lambda h=h, cell=cell: evac_part(h, cell))
                        deferred.append(
                            lambda h=h, cell=cell: recip_part(h, cell))
                        deferred.append(
                            lambda h=h, cell=cell: bc_part(h, cell))

                def defer_phase_c(b, tqb):
                    t0g = S * b + 512 * tqb
                    cell = {}

                    def oalloc():
                        cell["ot"] = otp.tile([128, 4, D], bf16, tag="ot",
                                              name=f"ot_{rep}_{b}_{tqb}")
                    for u in range(4):
                        for nb2 in range(2):
                            def th(u=u, nb2=nb2):
                                if u == 0 and nb2 == 0:
                                    oalloc()
                                op = mmp.tile([128, 512], f32, tag="mm",
                                              name=f"op_{rep}_{b}_{tqb}_{u}_{nb2}")
                                nc.tensor.matmul(
                                    op, vals[b][tqb][:, 128 * u:128 * u + 128],
                                    wo_sb[:, 512 * nb2:512 * nb2 + 512],
                                    start=True, stop=True)
                                nc.vector.tensor_copy(
                                    cell["ot"][:, u, 512 * nb2:512 * nb2 + 512],
                                    op)
                            deferred.append(th)

                    def dth():
                        nc.sync.dma_start(
                            out[t0g:t0g + 512, :].rearrange(
                                "(u p) f -> p u f", p=128), cell["ot"])
                    deferred.append(dth)

                def emit_tqb(b, tqb):
                    if b == 1 and tqb == 0:
                        # batch 1's attention reads qkt[1]/vext[1] produced by
                        # the deferred Phase A(b=1) thunks -- every one of
                        # them must be EMITTED (program order = dependency
                        # order for Tile) before these chunks are.
                        pop_def(len(deferred))
                    avs = [avp.tile([128, 512], f32, tag="av",
                                    name=f"av_{rep}_{b}_{tqb}_{h}")
                           for h in range(2)]
                    for c in range(NCH):
                        pt = emit_sc_exp(b, tqb, c)
                        emit_av(b, tqb, c, pt, avs)
                        pop_def(3)
                    emit_normalize(b, tqb, avs)
                    defer_phase_c(b, tqb)

                # ---- head: Phase A (b=0) interleaved with (0, tqb0):
                # per chunk, the vp group lands between the score matmuls and
                # the (exp-gated) AV matmuls so ScalarE starts at ~t_qk(tb0)
                # and the PE never queues a long vp burst ahead of scores.
                run_group(qk_group(0, 0, 0))
                run_group(qk_group(0, 0, 1))
                avs00 = [avp.tile([128, 512], f32, tag="av",
                                  name=f"av_{rep}_0_0_{h}")
                         for h in range(2)]
                for c in range(NCH):
                    if c % 4 == 0 and c > 0:
                        run_group(qk_group(0, c // 4, 0))
                        run_group(qk_group(0, c // 4, 1))
                    pt = emit_sc_exp(0, 0, c)
                    run_group(v_group(0, c // 4, c % 4))
                    emit_av(0, 0, c, pt, avs00)
                emit_normalize(0, 0, avs00)
                defer_phase_c(0, 0)
                # ---- steady state ----
                for b in range(B):
                    for tqb in range(NB):
                        if b == 0 and tqb == 0:
                            continue
                        if b == 0 and tqb == 1:
                            for tb in range(4):
                                emit_tb_deferred(1, tb)
                        emit_tqb(b, tqb)
                while deferred:
                    pop_def(8)
    nc.compile()
    return nc


def make_in_maps(mha_x, self_mask, w_qkv, b_qkv, w_out, b_out, use_mask):
    """Host-side sharding / layout prep. Returns (in_maps, host_bias)."""
    import ml_dtypes
    bf = np.dtype(ml_dtypes.bfloat16)
    x = np.asarray(mha_x, np.float32).reshape(T, D)
    xT_np = np.ascontiguousarray(x.T)                   # [D, T]
    # [128, 8(cc), B, S] with xTb[p, cc, b, t] = x[2048b + t, 128cc + p]
    xTb_np = np.ascontiguousarray(
        xT_np.reshape(8, 128, B, S).transpose(1, 0, 2, 3).astype(bf))
    scale = 1.0 / np.sqrt(np.float32(HD))               # 1/8
    wqkv = np.asarray(w_qkv, np.float32)
    bqkv = np.asarray(b_qkv, np.float32)
    wout = np.asarray(w_out, np.float32)
    bout = np.asarray(b_out, np.float32)

    # reference packs w_qkv rows as [H, (q,k,v), HD]: head h's q rows are
    # wqkv[192h:192h+64], k rows +64, v rows +128.
    wq_rows = lambda h: wqkv[192 * h:192 * h + 64, :]
    wk_rows = lambda h: wqkv[192 * h + 64:192 * h + 128, :]
    wv_rows = lambda h: wqkv[192 * h + 128:192 * h + 192, :]
    bq_of = lambda h: bqkv[192 * h:192 * h + 64]
    bk_of = lambda h: bqkv[192 * h + 64:192 * h + 128]
    bv_of = lambda h: bqkv[192 * h + 128:192 * h + 192]

    in_maps = []
    for c in range(NCORES):
        h0, h1 = 2 * c, 2 * c + 1
        wq = np.concatenate([wq_rows(h0), wq_rows(h1)], 0) * scale
        wk = np.concatenate([wk_rows(h0), wk_rows(h1)], 0)
        wvm = np.concatenate([wv_rows(h0), wv_rows(h1)], 0)
        wqkT = np.concatenate([wq, wk], 0).T            # [1024, 256]
        wvT = wvm.T                                     # [1024, 128]
        m = {
            "xTb": xTb_np,
            "wqk": np.ascontiguousarray(
                wqkT.reshape(8, 128, 256).transpose(1, 0, 2).astype(bf)),
            "wv": np.ascontiguousarray(
                wvT.reshape(8, 128, 128).transpose(1, 0, 2).astype(bf)),
            "wo": np.ascontiguousarray(
                wout[:, 128 * c:128 * c + 128].T.astype(bf)),
            "bqk": np.ascontiguousarray(
                np.stack([np.concatenate([bq_of(h0), bq_of(h1)]) * scale,
                          np.concatenate([bk_of(h0), bk_of(h1)])], 1)),
        }
        if use_mask:
            m["maskT"] = np.ascontiguousarray(
                np.asarray(self_mask, np.float32).transpose(0, 2, 1))
            m["ident"] = np.eye(128, dtype=np.float32)
        in_maps.append(m)

    b_v_full = np.concatenate([bv_of(h) for h in range(H)])
    host_bias = b_v_full @ wout.T + bout                # [D], exact
    return in_maps, host_bias


def kernel(**inputs):
    from concourse.bass_utils import run_bass_kernel_spmd
    self_mask = np.asarray(inputs["self_mask"], np.float32)
    use_mask = bool(np.any(self_mask))
    key = ("nc", use_mask)
    if key not in _CACHE:
        _CACHE[key] = build_nc(use_mask)
    nc = _CACHE[key]
    in_maps, host_bias = make_in_maps(
        inputs["mha_x"], self_mask, inputs["w_qkv"], inputs["b_qkv"],
        inputs["w_out"], inputs["b_out"], use_mask)
    res = run_bass_kernel_spmd(nc, in_maps, core_ids=list(range(NCORES)))
    acc = np.zeros((T, D), np.float32)
    for c in range(NCORES):
        acc += np.asarray(res.results[c]["out"], np.float32)
    acc += host_bias[None, :]
    return acc.reshape(B, S, D)


# revision 26
# speedup vs baseline: 1.2425x; 1.2406x over previous
"""MultiHeadAttention forward on 8 Trainium2 NeuronCores (Bass/Tile), v2.

Problem (hardcoded): B=2, S=2048, D=1024, H=16, HD=64.
  qkv = x @ w_qkv.T + b_qkv ; per-head attention with softmax(q k^T/8 + mask);
  out = values @ w_out.T + b_out.

Sharding: tensor-parallel over heads -- core c owns heads {2c, 2c+1}
(value dims 128c..128c+127).  Each core computes its 2 heads end-to-end and
a partial output projection in bf16; the host sums the 8 partials and adds
the bias constant (b_out + b_v @ w_out.T, exact because softmax rows sum
to 1).

v2 design notes (vs the v1 baseline this evolved from):
 - Everything is bf16 on the wire and in SBUF (x, q/k, v, probs, vals, w):
   halves DMA traffic and DVE element counts; matmuls run 1 cyc/row with
   FWL weight loads.  PSUM accumulation stays f32.
 - x is loaded once (16 x 512KB DMAs, batch-0 chunks first) and stays
   SBUF-resident; the ones row/columns come from memsets, not DMAs.
   Weight DMAs go out on the scalar-engine HWDGE queue in 4 batched
   transfers so the sync queue starts on x immediately (v1 spent ~70us
   serially issuing 65 tiny DMAs before any compute).
 - Scores are computed transposed (S^T = K^T.T @ Q^T per head) so exp runs
   on ScalarE straight out of PSUM; the two heads' score matmuls sit on
   disjoint PE row groups (partitions 0-63 / 64-127) and run concurrently.
 - V carries ones columns ([64v|32ones] for head 0, [32ones|64v] for head
   1, written at PSUM base 0 / 32) so one AV matmul per head yields both
   values^T and the softmax denominator l; head 1's values land on PSUM
   partitions 64:128, so the two heads' normalized values form a single
   [128, tq] vals tile and the output projection is one K=128 matmul per
   128-token block (v1 needed two K=64 matmuls).
 - l sits on a single partition row; a direct DVE reciprocal on the [1,512]
   row replaces v1's 32x32 transpose dance, and a K=1 matmul against a
   memset ones row broadcasts 1/l across partitions for the normalize
   multiply.
 - The emission order software-pipelines the whole kernel: Phase A (b=0)
   is interleaved with the first attention block's chunk loop, and a
   deferred-thunk queue drips Phase A (b=1) and the previous block's
   output projection into later chunk loops, so the PE never idles long
   enough for the HAM clock gate to re-throttle it to 1.2 GHz (v1 ran
   ~59% of its span at half clock) while ScalarE exp (the Phase B floor,
   ~1.15us per 128x1024 chunk) stays saturated.
"""
import sys
if "/opt/trn_rl_repo" not in sys.path:
    sys.path.insert(0, "/opt/trn_rl_repo")
import numpy as np

B, S, D, H = 2, 2048, 1024, 16
HD = D // H           # 64
NCORES = 8
T = B * S             # 4096 tokens
NB = S // 512         # 4 tq blocks per batch
NCH = S // 128        # 16 kpos chunks per batch

_CACHE = {}


def build_nc(use_mask: bool, reps: int = 1):
    """Build + compile the per-core Bass program (SPMD-identical)."""
    import concourse.bacc as bacc
    import concourse.tile as tile
    from concourse import mybir

    f32 = mybir.dt.float32
    f32r = mybir.dt.float32r
    bf16 = mybir.dt.bfloat16
    EXP = mybir.ActivationFunctionType.Exp
    MULT = mybir.AluOpType.mult

    nc = bacc.Bacc("TRN2", target_bir_lowering=False, debug=False,
                   num_devices=NCORES)

    xTb = nc.dram_tensor("xTb", (128, 8, B, S), bf16, kind="ExternalInput")
    wqk = nc.dram_tensor("wqk", (128, 8, 256), bf16, kind="ExternalInput")
    wv = nc.dram_tensor("wv", (128, 8, 128), bf16, kind="ExternalInput")
    wo = nc.dram_tensor("wo", (128, D), bf16, kind="ExternalInput")
    bqk = nc.dram_tensor("bqk", (128, 2), f32, kind="ExternalInput")
    if use_mask:
        maskT = nc.dram_tensor("maskT", (B, S, S), f32r, kind="ExternalInput")
        ident = nc.dram_tensor("ident", (128, 128), f32r, kind="ExternalInput")
    out = nc.dram_tensor("out", (T, D), bf16, kind="ExternalOutput")

    with tile.TileContext(nc) as tc:
        with tc.tile_pool(name="sbp", bufs=1) as sbp, \
             tc.tile_pool(name="ptp", bufs=4) as ptp, \
             tc.tile_pool(name="otp", bufs=2) as otp, \
             tc.tile_pool(name="rlp", bufs=6) as rlp, \
             tc.tile_pool(name="bcp", bufs=6) as bcp, \
             tc.tile_pool(name="mkp", bufs=4) as mkp, \
             tc.tile_pool(name="mmp", bufs=2, space="PSUM") as mmp, \
             tc.tile_pool(name="scp", bufs=2, space="PSUM") as scp, \
             tc.tile_pool(name="avp", bufs=2, space="PSUM") as avp:

            # --- persistent SBUF tensors (separate tiles per block so the
            # Tile dependency tracker never sees false cross-block deps) ---
            xb = [[sbp.tile([128, S], bf16, name=f"xb_{cc}_{b}")
                   for b in range(B)] for cc in range(8)]
            qkt = [[sbp.tile([128, 2, 512], bf16, name=f"qkt_{b}_{tb}")
                    for tb in range(4)] for b in range(B)]
            vext = [[sbp.tile([128, 2, 4, 128], bf16, name=f"vext_{b}_{tb}")
                     for tb in range(4)] for b in range(B)]
            vals = [[sbp.tile([128, 512], bf16, name=f"vals_{b}_{tqb}")
                     for tqb in range(NB)] for b in range(B)]
            wqk_sb = sbp.tile([128, 8, 256], bf16, name="wqk_sb")
            wv_sb = sbp.tile([128, 8, 128], bf16, name="wv_sb")
            wo_sb = sbp.tile([128, D], bf16, name="wo_sb")
            bqk_sb = sbp.tile([128, 2], f32, name="bqk_sb")
            ones_f32 = sbp.tile([65, 128], f32, name="ones_f32")
            ones_sb = sbp.tile([65, 128], f32r, name="ones_sb")
            if use_mask:
                id_sb = sbp.tile([128, 128], f32r, name="id_sb")

            # weight loads on the scalar-engine HWDGE queue (parallel with
            # the x loads below on the sync queue)
            nc.scalar.dma_start(wqk_sb, wqk[:, :, :])
            nc.scalar.dma_start(wv_sb, wv[:, :, :])
            nc.scalar.dma_start(wo_sb, wo[:, :])
            nc.scalar.dma_start(bqk_sb, bqk[:, :])
            if use_mask:
                nc.scalar.dma_start(id_sb, ident[:, :])

            nc.vector.memset(ones_f32, 1.0)
            nc.vector.tensor_copy(ones_sb, ones_f32)
            for b in range(B):
                for tb in range(4):
                    nc.vector.memset(vext[b][tb][:, 0, :, 64:128], 1.0)
                    nc.vector.memset(vext[b][tb][:, 1, :, 0:64], 1.0)

            for rep in range(reps):
                # x loads: batch 0's feature chunks first so Phase A can
                # start as soon as chunk (0,0) lands
                if rep == 0:
                    for b in range(B):
                        for cc in range(8):
                            nc.sync.dma_start(xb[cc][b], xTb[:, cc, b, :])

                deferred = []

                def pop_def(n):
                    for _ in range(min(n, len(deferred))):
                        deferred.pop(0)()

                def qk_group(b, tb, m):
                    t0 = 512 * tb
                    cell = {}

                    def start():
                        cell["acc"] = mmp.tile(
                            [128, 512], f32, tag="mm",
                            name=f"qk_{rep}_{b}_{tb}_{m}")

                    def mm(cc):
                        nc.tensor.matmul(
                            cell["acc"],
                            wqk_sb[:, cc, 128 * m:128 * m + 128],
                            xb[cc][b][:, t0:t0 + 512],
                            start=(cc == 0), stop=(cc == 7))

                    def fin():
                        nc.vector.tensor_scalar_add(
                            qkt[b][tb][:, m, :], cell["acc"],
                            bqk_sb[:, m:m + 1])
                    return start, mm, fin

                def v_group(b, tb, u):
                    t0 = 512 * tb
                    cell = {}

                    def start():
                        cell["vp"] = mmp.tile(
                            [128, 512], f32, tag="mm",
                            name=f"vp_{rep}_{b}_{tb}_{u}")

                    def mm(cc):
                        nc.tensor.matmul(
                            cell["vp"][:, 0:128],
                            xb[cc][b][:, t0 + 128 * u:t0 + 128 * u + 128],
                            wv_sb[:, cc, :],
                            start=(cc == 0), stop=(cc == 7))

                    def fin():
                        nc.vector.tensor_copy(
                            vext[b][tb][:, 0, u, 0:64], cell["vp"][:, 0:64])
                        nc.vector.tensor_copy(
                            vext[b][tb][:, 1, u, 64:128], cell["vp"][:, 64:128])
                    return start, mm, fin

                def run_group(g):
                    start, mm, fin = g
                    start()
                    for cc in range(8):
                        mm(cc)
                    fin()

                def emit_tb_deferred(b, tb):
                    """Phase A for one token block as small deferred thunks."""
                    groups = [qk_group(b, tb, 0), qk_group(b, tb, 1)] + \
                             [v_group(b, tb, u) for u in range(4)]
                    for (start, mm, fin) in groups:
                        for cc0 in range(0, 8, 2):
                            def th(start=start, mm=mm, cc0=cc0):
                                if cc0 == 0:
                                    start()
                                mm(cc0)
                                mm(cc0 + 1)
                            deferred.append(th)
                        deferred.append(fin)

                def emit_sc_exp(b, tqb, c):
                    sc = scp.tile([128, 1024], f32, tag="sc",
                                  name=f"sc_{rep}_{b}_{tqb}_{c}")
                    q_aps = [qkt[b][tqb][64 * h:64 * h + 64, 0, :]
                             for h in range(2)]
                    for h in range(2):
                        k_ap = qkt[b][c // 4][64 * h:64 * h + 64, 1,
                                             128 * (c % 4):128 * (c % 4) + 128]
                        nc.tensor.matmul(sc[:, 512 * h:512 * h + 512],
                                         k_ap, q_aps[h],
                                         start=True, stop=(not use_mask))
                    if use_mask:
                        mt = mkp.tile([128, 512], f32r, tag="mk",
                                      name=f"mk_{rep}_{b}_{tqb}_{c}")
                        nc.sync.dma_start(
                            mt, maskT[b, 128 * c:128 * c + 128,
                                      512 * tqb:512 * tqb + 512])
                        for h in range(2):
                            nc.tensor.matmul(sc[:, 512 * h:512 * h + 512],
                                             id_sb, mt, start=False, stop=True)
                    pt = ptp.tile([128, 1024], bf16, tag="pt",
                                  name=f"pt_{rep}_{b}_{tqb}_{c}")
                    nc.scalar.activation(pt, sc, EXP)
                    return pt

                def emit_av(b, tqb, c, pt, avs):
                    for h in range(2):
                        nc.tensor.matmul(avs[h][:, :],
                                         vext[b][c // 4][:, h, c % 4, :],
                                         pt[:, 512 * h:512 * h + 512],
                                         start=(c == 0), stop=(c == NCH - 1))

                def emit_normalize(b, tqb, avs):
                    """Deferred softmax normalization.  The first thunk copies
                    everything needed out of the PSUM av tiles (releasing
                    their slots for the next block's AV accumulation with no
                    PE->DVE->PE cycle).  1/l comes from a K=1 matmul that
                    broadcasts l across all 128 partitions followed by
                    reciprocal_approx_fast on the broadcast tile -- the op is
                    only correct at partition base 0, and partition count is
                    free on the DVE, so inverting the 128-row broadcast is
                    as cheap as inverting the single row."""
                    def evac_part(h, cell):
                        av = avs[h]
                        lrow = 64 if h == 0 else 32   # the denominator row
                        vlo = 0 if h == 0 else 64     # values partition base
                        avu = bcp.tile([128, 512], bf16, tag="avu",
                                       name=f"avu_{rep}_{b}_{tqb}_{h}")
                        nc.vector.tensor_copy(avu[vlo:vlo + 64, :],
                                              av[vlo:vlo + 64, :])
                        lsb = rlp.tile([65, 512], f32r, tag="lsb",
                                       name=f"lsb_{rep}_{b}_{tqb}_{h}")
                        nc.vector.tensor_copy(lsb[lrow:lrow + 1, :],
                                              av[lrow:lrow + 1, :])
                        cell["avu"], cell["lsb"] = avu, lsb

                    def recip_part(h, cell):
                        lrow = 64 if h == 0 else 32
                        bct = mmp.tile([128, 512], f32, tag="mm",
                                       name=f"bc_{rep}_{b}_{tqb}_{h}")
                        nc.tensor.matmul(bct[:, :],
                                         ones_sb[lrow:lrow + 1, :],
                                         cell["lsb"][lrow:lrow + 1, :],
                                         start=True, stop=True)
                        rlbc = bcp.tile([128, 512], f32, tag="rlbc",
                                        name=f"rlbc_{rep}_{b}_{tqb}_{h}")
                        nc.vector.reciprocal_approx_fast(out=rlbc, in_=bct[:, :])
                        cell["rlbc"] = rlbc

                    def mult_part(h, cell):
                        vlo = 0 if h == 0 else 64
                        nc.vector.tensor_tensor(
                            vals[b][tqb][vlo:vlo + 64, :],
                            cell["avu"][vlo:vlo + 64, :],
                            cell["rlbc"][vlo:vlo + 64, :], MULT)

                    for h in range(2):
                        cell = {}
                        deferred.append(
                            lambda h=h, cell=cell: evac_part(h, cell))
                        deferred.append(
                            lambda h=h, cell=cell: recip_part(h, cell))
                        deferred.append(
                            lambda h=h, cell=cell: mult_part(h, cell))

                def defer_phase_c(b, tqb):
                    t0g = S * b + 512 * tqb
                    cell = {}

                    def oalloc():
                        cell["ot"] = otp.tile([128, 4, D], bf16, tag="ot",
                                              name=f"ot_{rep}_{b}_{tqb}")
                    for u in range(4):
                        for nb2 in range(2):
                            def th(u=u, nb2=nb2):
                                if u == 0 and nb2 == 0:
                                    oalloc()
                                op = mmp.tile([128, 512], f32, tag="mm",
                                              name=f"op_{rep}_{b}_{tqb}_{u}_{nb2}")
                                nc.tensor.matmul(
                                    op, vals[b][tqb][:, 128 * u:128 * u + 128],
                                    wo_sb[:, 512 * nb2:512 * nb2 + 512],
                                    start=True, stop=True)
                                nc.vector.tensor_copy(
                                    cell["ot"][:, u, 512 * nb2:512 * nb2 + 512],
                                    op)
                            deferred.append(th)

                    def dth():
                        nc.sync.dma_start(
                            out[t0g:t0g + 512, :].rearrange(
                                "(u p) f -> p u f", p=128), cell["ot"])
                    deferred.append(dth)

                def emit_tqb(b, tqb):
                    if b == 1 and tqb == 0:
                        # batch 1's attention reads qkt[1]/vext[1] produced by
                        # the deferred Phase A(b=1) thunks -- every one of
                        # them must be EMITTED (program order = dependency
                        # order for Tile) before these chunks are.
                        pop_def(len(deferred))
                    avs = [avp.tile([128, 512], f32, tag="av",
                                    name=f"av_{rep}_{b}_{tqb}_{h}")
                           for h in range(2)]
                    for c in range(NCH):
                        pt = emit_sc_exp(b, tqb, c)
                        emit_av(b, tqb, c, pt, avs)
                        pop_def(3)
                    emit_normalize(b, tqb, avs)
                    defer_phase_c(b, tqb)

                # ---- head: Phase A (b=0) interleaved with (0, tqb0):
                # per chunk, the vp group lands between the score matmuls and
                # the (exp-gated) AV matmuls so ScalarE starts at ~t_qk(tb0)
                # and the PE never queues a long vp burst ahead of scores.
                run_group(qk_group(0, 0, 0))
                run_group(qk_group(0, 0, 1))
                avs00 = [avp.tile([128, 512], f32, tag="av",
                                  name=f"av_{rep}_0_0_{h}")
                         for h in range(2)]
                for c in range(NCH):
                    if c % 4 == 0 and c > 0:
                        run_group(qk_group(0, c // 4, 0))
                        run_group(qk_group(0, c // 4, 1))
                    pt = emit_sc_exp(0, 0, c)
                    run_group(v_group(0, c // 4, c % 4))
                    emit_av(0, 0, c, pt, avs00)
                emit_normalize(0, 0, avs00)
                defer_phase_c(0, 0)
                # ---- steady state ----
                for b in range(B):
                    for tqb in range(NB):
                        if b == 0 and tqb == 0:
                            continue
                        if b == 0 and tqb == 1:
                            for tb in range(4):
                                emit_tb_deferred(1, tb)
                        emit_tqb(b, tqb)
                while deferred:
                    pop_def(8)
    nc.compile()
    return nc


def make_in_maps(mha_x, self_mask, w_qkv, b_qkv, w_out, b_out, use_mask):
    """Host-side sharding / layout prep. Returns (in_maps, host_bias)."""
    import ml_dtypes
    bf = np.dtype(ml_dtypes.bfloat16)
    x = np.asarray(mha_x, np.float32).reshape(T, D)
    xT_np = np.ascontiguousarray(x.T)                   # [D, T]
    # [128, 8(cc), B, S] with xTb[p, cc, b, t] = x[2048b + t, 128cc + p]
    xTb_np = np.ascontiguousarray(
        xT_np.reshape(8, 128, B, S).transpose(1, 0, 2, 3).astype(bf))
    scale = 1.0 / np.sqrt(np.float32(HD))               # 1/8
    wqkv = np.asarray(w_qkv, np.float32)
    bqkv = np.asarray(b_qkv, np.float32)
    wout = np.asarray(w_out, np.float32)
    bout = np.asarray(b_out, np.float32)

    # reference packs w_qkv rows as [H, (q,k,v), HD]: head h's q rows are
    # wqkv[192h:192h+64], k rows +64, v rows +128.
    wq_rows = lambda h: wqkv[192 * h:192 * h + 64, :]
    wk_rows = lambda h: wqkv[192 * h + 64:192 * h + 128, :]
    wv_rows = lambda h: wqkv[192 * h + 128:192 * h + 192, :]
    bq_of = lambda h: bqkv[192 * h:192 * h + 64]
    bk_of = lambda h: bqkv[192 * h + 64:192 * h + 128]
    bv_of = lambda h: bqkv[192 * h + 128:192 * h + 192]

    in_maps = []
    for c in range(NCORES):
        h0, h1 = 2 * c, 2 * c + 1
        wq = np.concatenate([wq_rows(h0), wq_rows(h1)], 0) * scale
        wk = np.concatenate([wk_rows(h0), wk_rows(h1)], 0)
        wvm = np.concatenate([wv_rows(h0), wv_rows(h1)], 0)
        wqkT = np.concatenate([wq, wk], 0).T            # [1024, 256]
        wvT = wvm.T                                     # [1024, 128]
        m = {
            "xTb": xTb_np,
            "wqk": np.ascontiguousarray(
                wqkT.reshape(8, 128, 256).transpose(1, 0, 2).astype(bf)),
            "wv": np.ascontiguousarray(
                wvT.reshape(8, 128, 128).transpose(1, 0, 2).astype(bf)),
            "wo": np.ascontiguousarray(
                wout[:, 128 * c:128 * c + 128].T.astype(bf)),
            "bqk": np.ascontiguousarray(
                np.stack([np.concatenate([bq_of(h0), bq_of(h1)]) * scale,
                          np.concatenate([bk_of(h0), bk_of(h1)])], 1)),
        }
        if use_mask:
            m["maskT"] = np.ascontiguousarray(
                np.asarray(self_mask, np.float32).transpose(0, 2, 1))
            m["ident"] = np.eye(128, dtype=np.float32)
        in_maps.append(m)

    b_v_full = np.concatenate([bv_of(h) for h in range(H)])
    host_bias = b_v_full @ wout.T + bout                # [D], exact
    return in_maps, host_bias


def kernel(**inputs):
    from concourse.bass_utils import run_bass_kernel_spmd
    self_mask = np.asarray(inputs["self_mask"], np.float32)
    use_mask = bool(np.any(self_mask))
    key = ("nc", use_mask)
    if key not in _CACHE:
        _CACHE[key] = build_nc(use_mask)
    nc = _CACHE[key]
    in_maps, host_bias = make_in_maps(
        inputs["mha_x"], self_mask, inputs["w_qkv"], inputs["b_qkv"],
        inputs["w_out"], inputs["b_out"], use_mask)
    res = run_bass_kernel_spmd(nc, in_maps, core_ids=list(range(NCORES)))
    acc = np.zeros((T, D), np.float32)
    for c in range(NCORES):
        acc += np.asarray(res.results[c]["out"], np.float32)
    acc += host_bias[None, :]
    return acc.reshape(B, S, D)
